# Initial kernel scaffold
#
"""Trainium2 Bass kernel for nn_CLRerHead (CLRNet-style lane-detection head).

Sharding: data-parallel over batch. 32 batch items -> 8 cores x 4 items each.
Each core runs the full 3-level refinement for its 4 items.

Gather strategy ("negative tent" matmul gather): per level,
  grid_sample + roi-flatten + FC fuse into two PE matmul stages:
    RCPROJ_s (W, 64) = sum_{corner} featRow_y(64, W).T @ (-wy*Wfc_s)(64, 64)
    f (64, 192)     += sum_s matmul(lhsT=RCPROJ_s(W, 64), rhs=v_s(W, 192))
  with v(W, 192*36) = min(|broadcast(xf) - iota| - 1, 0) = -(x-direction tent
  weight), built by gpsimd partition_broadcast + ACT Abs + DVE tensor_scalar.
  The two minus signs cancel.
"""

import math
import numpy as np
import ml_dtypes
from contextlib import ExitStack

import concourse.bass as bass
import concourse.bacc as bacc
import concourse.mybir as mybir
import concourse.tile as tile
from concourse import bass_utils

dt = mybir.dt
AF = mybir.ActivationFunctionType
ALU = mybir.AluOpType

# ---------------- static problem config ----------------
IMG_W, IMG_H = 800.0, 320.0
NR, NS, NP, FC = 72, 36, 192, 64
N_STRIPS = NR - 1
ALPHA = IMG_H / IMG_W
SAMPLE_IDX = (np.linspace(0.0, 1.0, NS) * N_STRIPS).astype(np.int64)
PRIOR_FEAT_YS = np.flip(SAMPLE_IDX.astype(np.float32) / N_STRIPS).copy()
PRIOR_YS = np.linspace(1.0, 0.0, NR, dtype=np.float32)

N_CORES = 8
NB = 4
LEVELS = [(10, 25), (20, 50), (40, 100)]   # processing order: feat2, feat1, feat0
PCH = [(0, 128), (128, 64)]
BF16 = dt.bfloat16
FP16 = dt.float16
F32 = dt.float32
NPTS = NP * NS
NHALF = NPTS // 2
NQ = NPTS // 4
SPQ = NS // 4
SPH = NS // 2
SG = 8

Q_S = (1.0 - PRIOR_YS[SAMPLE_IDX[::-1]]).astype(np.float32)
QF_R = (1.0 - PRIOR_YS).astype(np.float32)


def _level_ytab(H):
    ys = PRIOR_FEAT_YS * (H - 1)
    y0 = np.clip(np.floor(ys).astype(np.int64), 0, H - 1)
    y1 = np.minimum(y0 + 1, H - 1)
    wy1 = (ys - y0).astype(np.float32)
    wy1 = np.where(y1 == y0, 0.0, wy1).astype(np.float32)
    wy0 = (1.0 - wy1).astype(np.float32)
    return y0, y1, wy0, wy1


def _proj_descriptors(H):
    """[(corner, y, sa, sb, col, first, last)] with one start/stop per 8-s group."""
    y0, y1, _, _ = _level_ytab(H)
    raw = []
    for corner, yc in enumerate((y0, y1)):
        s = 0
        while s < NS:
            e = s
            while e + 1 < NS and yc[e + 1] == yc[s]:
                e += 1
            a = s
            while a <= e:
                b = min(e, (a // SG) * SG + SG - 1)
                raw.append((corner, int(yc[s]), a, b))
                a = b + 1
            s = e + 1
    raw.sort(key=lambda r: (r[2] // SG, r[0], r[2]))
    descs, col, seen, last_in_group = [], 0, set(), {}
    for i, (corner, y, sa, sb) in enumerate(raw):
        g = sa // SG
        descs.append([corner, y, sa, sb, col, g not in seen, False])
        seen.add(g)
        last_in_group[g] = i
        col += 64 * (sb - sa + 1)
    for i in last_in_group.values():
        descs[i][6] = True
    return descs, col


def _neg_wywfc(W_fc, H):
    _, _, wy0, wy1 = _level_ytab(H)
    wyc = [wy0, wy1]
    descs, ncols = _proj_descriptors(H)
    out = np.zeros((64, ncols), np.float32)
    for corner, y, sa, sb, col, _, _ in descs:
        for s in range(sa, sb + 1):
            out[:, col:col + 64] = -wyc[corner][s] * W_fc[s::NS, :]
            col += 64
    return out.astype(np.float16)


_CACHE = {}


def _build_program(num_devices=N_CORES):
    nc = bacc.Bacc("TRN2", target_bir_lowering=False, debug=False,
                   num_devices=num_devices)
    D = {}

    def din(name, shape, dtype=F32):
        D[name] = nc.dram_tensor(name, list(shape), dtype, kind="ExternalInput")

    for li, (H, W) in enumerate(LEVELS):
        din(f"featbf{li}", (64, NB * H * W), FP16)
        _, ncols = _proj_descriptors(H)
        din(f"nwfc{li}", (64, ncols), FP16)
    din("anch", (NB, NP, 3))
    din("sinargsT", (64, NB))
    din("W_t1", (64, 256)); din("b_t1", (128, 2))
    din("W_t2a", (128, 256)); din("W_t2b", (128, 256)); din("b_t2", (128, 2))
    din("W_sta", (128, 128)); din("W_stb", (128, 128))
    din("bstS1", (64, 1)); din("bstSh", (64, 1))
    din("W_tca", (128, 64)); din("W_tcb", (128, 64)); din("b_tc", (64, 1))
    for w in ["W_q", "W_k", "W_v", "W_c1", "W_c2", "W_r1", "W_r2"]:
        din(w, (64, 64))
    din("W_o_bf", (64, 64), FP16)
    din("W_cls", (64, 2)); din("W_reg", (64, 76))
    for bnm in ["b_fc", "b_c1", "b_c2", "b_r1", "b_r2"]:
        din(bnm, (64, 1))
    din("b_cls", (2, 1)); din("b_reg", (76, 1))
    din("qrep", (128, NS)); din("qfrep", (128, NR))
    din("negiota", (128, 1)); din("halfpi", (128, 1)); din("ident", (128, 128))
    din("ones_bf", (128, 1), FP16)

    out_t = nc.dram_tensor("out", [NB, NP, 78], F32, kind="ExternalOutput")

    with tile.TileContext(nc) as tc, ExitStack() as ex:
        cpool = ex.enter_context(tc.tile_pool(name="consts", bufs=1))
        state = ex.enter_context(tc.tile_pool(name="state", bufs=1))
        wk = ex.enter_context(tc.tile_pool(name="work", bufs=2))
        big = ex.enter_context(tc.tile_pool(name="big", bufs=2))
        ps = ex.enter_context(tc.tile_pool(name="ps", bufs=4, space="PSUM"))
        psf = ex.enter_context(tc.tile_pool(name="psf", bufs=2, space="PSUM"))
        psrc = ex.enter_context(tc.tile_pool(name="psrc", bufs=2, space="PSUM"))

        C = {}
        for name, t in D.items():
            if name == "anch":
                continue
            C[name] = cpool.tile(list(t.shape), t.dtype, tag=name, name=f"c_{name}")
            nc.sync.dma_start(C[name][:], t.ap())

        anch = {}
        for b in range(NB):
            for ci, (p0, pn) in enumerate(PCH):
                a = state.tile([pn, 3], F32, tag=f"anch{b}_{ci}", name=f"anch{b}_{ci}")
                nc.sync.dma_start(a[:], D["anch"].ap()[b, p0:p0 + pn, :])
                anch[(b, ci)] = a

        # ---------------- time MLP ----------------
        sinT = wk.tile([64, NB], F32, tag="tm_sin", name="sinT")
        nc.scalar.activation(sinT[:], C["sinargsT"][:], AF.Sin)
        emb = []
        for m in range(2):
            p = ps.tile([128, NB], F32, tag="mm", name=f"p_emb{m}")
            nc.tensor.matmul(p[:], C["W_t1"][:, m * 128:(m + 1) * 128], sinT[:])
            x = state.tile([128, NB], F32, tag=f"emb{m}", name=f"emb{m}")
            nc.scalar.activation(x[:], p[:], AF.Identity, bias=C["b_t1"][:, m:m + 1])
            sq = wk.tile([128, NB], F32, tag="tm_sq", name=f"sq{m}")
            nc.scalar.activation(sq[:], x[:], AF.Square)
            cu = wk.tile([128, NB], F32, tag="tm_cu", name=f"cu{m}")
            nc.vector.tensor_tensor(cu[:], sq[:], x[:], ALU.mult)
            nc.vector.tensor_scalar(cu[:], cu[:], 0.044715, None, ALU.mult)
            nc.vector.tensor_tensor(cu[:], cu[:], x[:], ALU.add)
            th = wk.tile([128, NB], F32, tag="tm_th", name=f"th{m}")
            nc.scalar.activation(th[:], cu[:], AF.Tanh,
                                 scale=float(np.sqrt(2.0 / np.pi)))
            nc.vector.tensor_scalar(th[:], th[:], 1.0, 0.5, ALU.add, ALU.mult)
            nc.vector.tensor_tensor(x[:], th[:], x[:], ALU.mult)
            emb.append(x)
        tmb = []
        for m in range(2):
            p = ps.tile([128, NB], F32, tag="mm", name=f"p_tmb{m}")
            for k in range(2):
                wt2 = C["W_t2a"] if k == 0 else C["W_t2b"]
                nc.tensor.matmul(p[:], wt2[:, m * 128:(m + 1) * 128], emb[k][:],
                                 start=(k == 0), stop=(k == 1))
            x = state.tile([128, NB], F32, tag=f"tmb{m}", name=f"tmb{m}")
            nc.scalar.activation(x[:], p[:], AF.Identity, bias=C["b_t2"][:, m:m + 1])
            tmb.append(x)
        sil = []
        for m in range(2):
            s = wk.tile([128, NB], F32, tag=f"tm_sil{m}", name=f"sil{m}")
            nc.scalar.activation(s[:], tmb[m][:], AF.Sigmoid)
            nc.vector.tensor_tensor(s[:], s[:], tmb[m][:], ALU.mult)
            sil.append(s)
        scale1T = state.tile([64, NB], F32, tag="scale1T", name="scale1T")
        shiftT = state.tile([64, NB], F32, tag="shiftT", name="shiftT")
        for j, (dst, bias) in enumerate([(scale1T, "bstS1"), (shiftT, "bstSh")]):
            p = ps.tile([64, NB], F32, tag="mm", name=f"p_ss{j}")
            for k in range(2):
                wst = C["W_sta"] if k == 0 else C["W_stb"]
                nc.tensor.matmul(p[:], wst[:, j * 64:(j + 1) * 64], sil[k][:],
                                 start=(k == 0), stop=(k == 1))
            nc.scalar.activation(dst[:], p[:], AF.Identity, bias=C[bias][:, 0:1])
        tokT = state.tile([64, NB], F32, tag="tokT", name="tokT")
        ptk = ps.tile([64, NB], F32, tag="mm", name="p_tok")
        for k in range(2):
            wtc = C["W_tca"] if k == 0 else C["W_tcb"]
            nc.tensor.matmul(ptk[:], wtc[:], tmb[k][:], start=(k == 0), stop=(k == 1))
        nc.scalar.activation(tokT[:], ptk[:], AF.Identity, bias=C["b_tc"][:, 0:1])

        # ---------------- helpers ----------------
        def gen_ab(b, scaleW, tagsfx):
            """Emit trig + affine ops; return [(aC, bC, base, g)] per chunk."""
            res = []
            for ci, (p0, pn) in enumerate(PCH):
                A = anch[(b, ci)]
                sn = wk.tile([pn, 1], F32, tag=f"sn{ci}{tagsfx}", name=f"sn{b}{ci}")
                cs = wk.tile([pn, 1], F32, tag=f"cs{ci}{tagsfx}", name=f"cs{b}{ci}")
                nc.scalar.activation(sn[:], A[:, 2:3], AF.Sin, scale=math.pi)
                nc.scalar.activation(cs[:], A[:, 2:3], AF.Sin, scale=-math.pi,
                                     bias=C["halfpi"][0:pn, 0:1])
                g = wk.tile([pn, 1], F32, tag=f"g{ci}{tagsfx}", name=f"g{b}{ci}")
                nc.vector.reciprocal(g[:], sn[:])
                nc.vector.tensor_tensor(g[:], cs[:], g[:], ALU.mult)
                nc.vector.tensor_scalar(g[:], g[:], 1000.0, -1000.0,
                                        ALU.min, ALU.max)
                nc.vector.tensor_scalar(g[:], g[:], ALPHA, None, ALU.mult)
                base = wk.tile([pn, 1], F32, tag=f"bs{ci}{tagsfx}", name=f"bs{b}{ci}")
                nc.vector.tensor_tensor(base[:], A[:, 0:1], g[:], ALU.mult)
                nc.vector.tensor_tensor(base[:], A[:, 1:2], base[:], ALU.subtract)
                aC = wk.tile([pn, 1], F32, tag=f"aC{ci}{tagsfx}", name=f"aC{b}{ci}")
                bC = wk.tile([pn, 1], F32, tag=f"bC{ci}{tagsfx}", name=f"bC{b}{ci}")
                nc.vector.tensor_scalar(aC[:], base[:], scaleW, None, ALU.mult)
                nc.vector.tensor_scalar(bC[:], g[:], scaleW, None, ALU.mult)
                res.append((aC, bC, base, g))
            return res

        # ---------------- main loop ----------------
        for li, (H, W) in enumerate(LEVELS):
            HW = H * W
            descs, _ = _proj_descriptors(H)
            is_last = li == len(LEVELS) - 1
            feat = C[f"featbf{li}"]
            nwfc = C[f"nwfc{li}"]

            # stage A (trig table) for all b
            xfs = {}
            for b in range(NB):
                ab = gen_ab(b, float(W - 1), "a")
                for ci, (p0, pn) in enumerate(PCH):
                    aC, bC, _, _ = ab[ci]
                    xf = state.tile([pn, NS], F32, tag=f"xf{b}_{ci}",
                                    name=f"xf{b}_{ci}_{li}")
                    nc.vector.tensor_scalar(xf[:], C["qrep"][0:pn, :],
                                            bC[:, 0:1], aC[:, 0:1],
                                            ALU.mult, ALU.add)
                    xfs[(b, ci)] = xf

            for b in range(NB):
                # xf -> row (1, 6912) via PE transpose + DMA collapse
                xfTs = wk.tile([NS, NP], F32, tag="xfT", bufs=3, name=f"xfT{b}")
                for ci, (p0, pn) in enumerate(PCH):
                    pt = ps.tile([NS, 128], F32, tag="mm", name=f"p_xfT{b}{ci}")
                    nc.tensor.transpose(pt[:, 0:pn], xfs[(b, ci)][:],
                                        C["ident"][0:pn, 0:pn])
                    nc.vector.tensor_copy(xfTs[:, p0:p0 + pn], pt[:, 0:pn])
                vq = []
                for h in range(4):
                    xfrow = wk.tile([1, NQ], F32, tag="xfrow", bufs=3,
                                    name=f"xfrow{b}{h}")
                    nc.sync.dma_start(xfrow[0:1, :], xfTs[h * SPQ:(h + 1) * SPQ, :])
                    xfB = big.tile([128, NQ], F32, tag="xfB", bufs=2,
                                   name=f"xfB{b}{h}")
                    nc.gpsimd.partition_broadcast(
                        xfB[0:W, :], xfrow[0:1, :],
                        channels=W)
                    d1 = big.tile([128, NQ], F32, tag="d1", bufs=2,
                                  name=f"d1{b}{h}")
                    nc.scalar.activation(d1[0:W, :], xfB[0:W, :], AF.Abs,
                                         bias=C["negiota"][0:W, 0:1])
                    v = big.tile([128, NQ], F32, tag="vq", bufs=3, name=f"v{b}{h}")
                    nc.vector.tensor_scalar(v[0:W, :], d1[0:W, :], 1.0, 0.0,
                                            ALU.subtract, ALU.min)
                    vq.append(v)

                # PROJ for all s-groups first (PE work independent of tents)
                fps = psf.tile([64, NP], F32, tag="f_ps", name=f"fps{b}")
                rcs = []
                for g0 in range(0, NS, SG):
                    rc = psrc.tile([128, SG * 64], F32, tag="rc_ps",
                                   name=f"rc{b}_{g0}")
                    for corner, y, sa, sb, col, first, last in descs:
                        if sa // SG != g0 // SG:
                            continue
                        n = (sb - sa + 1) * 64
                        nc.tensor.matmul(
                            rc[0:W, (sa - g0) * 64:(sa - g0) * 64 + n],
                            feat[:, b * HW + y * W: b * HW + (y + 1) * W],
                            nwfc[:, col:col + n],
                            start=first, stop=last)
                    ng = (min(g0 + SG, NS) - g0) * 64
                    rcsb = wk.tile([128, SG * 64], F32, tag="rcsb", bufs=6,
                                   name=f"rcsb{b}_{g0}")
                    nc.vector.tensor_copy(rcsb[0:W, 0:ng], rc[0:W, 0:ng])
                    rcs.append(rcsb)
                for s in range(NS):
                    g0 = (s // SG) * SG
                    h = s // SPQ
                    nc.tensor.matmul(
                        fps[:],
                        rcs[s // SG][0:W, (s - g0) * 64:(s - g0 + 1) * 64],
                        vq[h][0:W, (s - SPQ * h) * NP:(s - SPQ * h + 1) * NP],
                        start=(s == 0), stop=(s == NS - 1))

                fT = wk.tile([64, NP], F32, tag="fT", bufs=3, name=f"fT{b}")
                nc.scalar.activation(fT[:], fps[:], AF.Relu, bias=C["b_fc"][:, 0:1])
                nc.vector.tensor_scalar(fT[:], fT[:], tokT[:, b:b + 1], None,
                                        ALU.add)

                # attention
                qp = ps.tile([64, NP], F32, tag="mm", name=f"qp{b}")
                nc.tensor.matmul(qp[:], C["W_q"][:], fT[:])
                qT = wk.tile([64, NP], F32, tag="qT", name=f"qT{b}")
                nc.scalar.activation(qT[:], qp[:], AF.Copy, scale=0.125)
                kp = ps.tile([64, NP], F32, tag="mm", name=f"kp{b}")
                nc.tensor.matmul(kp[:], C["W_k"][:], fT[:])
                kT = wk.tile([64, NP], F32, tag="kT", name=f"kT{b}")
                nc.vector.tensor_copy(kT[:], kp[:])
                vn = []
                for ci, (p0, pn) in enumerate(PCH):
                    vp = ps.tile([128, 64], F32, tag="mm", name=f"vp{b}{ci}")
                    nc.tensor.matmul(vp[0:pn, :], fT[:, p0:p0 + pn], C["W_v"][:])
                    vt = wk.tile([pn, 64], FP16, tag=f"vn{ci}", name=f"vn{b}{ci}")
                    nc.vector.tensor_copy(vt[:], vp[0:pn, :])
                    vn.append(vt)
                est = []
                for ci, (p0, pn) in enumerate(PCH):
                    sp = ps.tile([128, NP], F32, tag="mm", name=f"sp{b}{ci}")
                    nc.tensor.matmul(sp[0:pn, :], kT[:, p0:p0 + pn], qT[:])
                    e = wk.tile([pn, NP], FP16, tag=f"est{ci}", name=f"est{b}{ci}")
                    nc.scalar.activation(e[:], sp[0:pn, :], AF.Exp)
                    est.append(e)
                zp = ps.tile([1, NP], F32, tag="mm", name=f"zp{b}")
                for ci, (p0, pn) in enumerate(PCH):
                    nc.tensor.matmul(zp[:], C["ones_bf"][0:pn, 0:1], est[ci][:],
                                     start=(ci == 0), stop=(ci == 1))
                rrow = wk.tile([1, NP], F32, tag="rrow", name=f"rrow{b}")
                nc.vector.reciprocal(rrow[:], zp[:])
                rbc = wk.tile([64, NP], F32, tag="rbc", name=f"rbc{b}")
                nc.gpsimd.partition_broadcast(rbc[:], rrow[0:1, :], channels=64)
                avp = ps.tile([64, NP], F32, tag="mm", name=f"avp{b}")
                for ci in range(2):
                    nc.tensor.matmul(avp[:], vn[ci][:], est[ci][:],
                                     start=(ci == 0), stop=(ci == 1))
                avsb = wk.tile([64, NP], FP16, tag="avsb", name=f"avsb{b}")
                nc.vector.tensor_copy(avsb[:], avp[:])
                opp = ps.tile([64, NP], F32, tag="mm", name=f"opp{b}")
                nc.tensor.matmul(opp[:], C["W_o_bf"][:], avsb[:])
                t1 = wk.tile([64, NP], F32, tag="attnt", name=f"t1{b}")
                nc.vector.tensor_tensor(t1[:], opp[:], rbc[:], ALU.mult)
                nc.vector.tensor_tensor(fT[:], fT[:], t1[:], ALU.add)

                # FiLM
                nc.vector.tensor_scalar(fT[:], fT[:], scale1T[:, b:b + 1],
                                        shiftT[:, b:b + 1], ALU.mult, ALU.add)

                def head_mm(wname, bias, src, relu=True, out_p=64):
                    p = ps.tile([128, NP], F32, tag="mm", name=f"p_{wname}{b}")
                    nc.tensor.matmul(p[0:out_p, :], C[wname][:], src[:])
                    o = wk.tile([out_p, NP], F32, tag=f"hd_{wname}",
                                name=f"{wname}o{b}")
                    nc.scalar.activation(o[:], p[0:out_p, :],
                                         AF.Relu if relu else AF.Identity,
                                         bias=C[bias][:, 0:1])
                    return o

                r1 = head_mm("W_r1", "b_r1", fT)
                r2 = head_mm("W_r2", "b_r2", r1)
                regT = head_mm("W_reg", "b_reg", r2, relu=False, out_p=76)

                for ci, (p0, pn) in enumerate(PCH):
                    pt = ps.tile([128, 76], F32, tag="mm", name=f"p_rt{b}{ci}")
                    nc.tensor.transpose(pt[0:pn, :], regT[:, p0:p0 + pn],
                                        C["ident"][0:76, 0:76])
                    rn = state.tile([pn, 76], F32, tag=f"regn{b}_{ci}",
                                    name=f"regn{b}_{ci}_{li}")
                    nc.vector.tensor_copy(rn[:], pt[0:pn, :])
                    A = anch[(b, ci)]
                    nc.vector.tensor_tensor(A[:, :], A[:, :], rn[:, 0:3], ALU.add)
                    if is_last:
                        o = out_t.ap()
                        nc.sync.dma_start(o[b, p0:p0 + pn, 2:5], A[:])
                        nc.sync.dma_start(o[b, p0:p0 + pn, 5:6], rn[:, 3:4])
                        _CACHE.setdefault("regn", {})[(b, ci)] = rn

                if is_last:
                    c1 = head_mm("W_c1", "b_c1", fT)
                    c2 = head_mm("W_c2", "b_c2", c1)
                    clsT = head_mm("W_cls", "b_cls", c2, relu=False, out_p=2)
                    for ci, (p0, pn) in enumerate(PCH):
                        pt = ps.tile([128, 2], F32, tag="mm", name=f"p_ct{b}{ci}")
                        nc.tensor.transpose(pt[0:pn, :], clsT[:, p0:p0 + pn],
                                            C["ident"][0:2, 0:2])
                        cn = wk.tile([pn, 2], F32, tag=f"clsn{ci}",
                                     name=f"clsn{b}{ci}")
                        nc.vector.tensor_copy(cn[:], pt[0:pn, :])
                        nc.sync.dma_start(out_t.ap()[b, p0:p0 + pn, 0:2], cn[:])

            if is_last:
                # pred_xs from updated anchors (batched so Sin acts group)
                for b in range(NB):
                    ab = gen_ab(b, 1.0, "o")
                    for ci, (p0, pn) in enumerate(PCH):
                        _, _, base, g = ab[ci]
                        rn = _CACHE["regn"][(b, ci)]
                        xsf = wk.tile([pn, NR], F32, tag=f"xsf{ci}",
                                      name=f"xsf{b}{ci}")
                        nc.vector.tensor_scalar(xsf[:], C["qfrep"][0:pn, :],
                                                g[:, 0:1], base[:, 0:1],
                                                ALU.mult, ALU.add)
                        nc.vector.tensor_tensor(xsf[:], xsf[:], rn[:, 4:76],
                                                ALU.add)
                        nc.sync.dma_start(out_t.ap()[b, p0:p0 + pn, 6:78], xsf[:])

    nc.compile()
    _CACHE.pop("regn", None)
    return nc


def _host_inputs(inp_slice, nwfc):
    m = {}
    for li, key in enumerate(["feat2", "feat1", "feat0"]):
        f = np.asarray(inp_slice[key], np.float32)
        H, W = LEVELS[li]
        fb = np.ascontiguousarray(f.transpose(1, 0, 2, 3).reshape(64, NB * H * W))
        m[f"featbf{li}"] = fb.astype(np.float16)
        m[f"nwfc{li}"] = nwfc[li]
    m["anch"] = np.asarray(inp_slice["inputs"], np.float32)
    half = FC // 2
    freqs = np.exp(np.arange(half, dtype=np.float32)
                   * (-math.log(10000.0) / (half - 1)))
    ang = np.asarray(inp_slice["t"]).astype(np.float32)[:, None] * freqs[None, :]
    full = np.concatenate([ang, ang + math.pi / 2.0], axis=1)
    full = np.mod(full + math.pi, 2.0 * math.pi) - math.pi
    m["sinargsT"] = np.ascontiguousarray(full.T).astype(np.float32)
    w = {k: np.asarray(v, np.float32) for k, v in inp_slice.items()
         if k.startswith(("W_", "b_"))}
    m["W_t1"] = w["W_t1"]
    m["b_t1"] = np.ascontiguousarray(w["b_t1"].reshape(2, 128).T)
    m["W_t2a"] = w["W_t2"][:128]; m["W_t2b"] = w["W_t2"][128:]
    m["b_t2"] = np.ascontiguousarray(w["b_t2"].reshape(2, 128).T)
    m["W_sta"] = w["W_st"][:128]; m["W_stb"] = w["W_st"][128:]
    m["bstS1"] = (w["b_st"][:64] + 1.0).reshape(-1, 1)
    m["bstSh"] = w["b_st"][64:].reshape(-1, 1)
    m["W_tca"] = w["W_tc"][:128]; m["W_tcb"] = w["W_tc"][128:]
    m["b_tc"] = w["b_tc"].reshape(-1, 1)
    for k in ["W_q", "W_k", "W_v", "W_c1", "W_c2", "W_r1", "W_r2",
              "W_cls", "W_reg"]:
        m[k] = w[k]
    m["W_o_bf"] = w["W_o"].astype(np.float16)
    for k in ["b_fc", "b_c1", "b_c2", "b_r1", "b_r2", "b_cls", "b_reg"]:
        m[k] = w[k].reshape(-1, 1)
    m["qrep"] = np.broadcast_to(Q_S[None, :], (128, NS)).copy()
    m["qfrep"] = np.broadcast_to(QF_R[None, :], (128, NR)).copy()
    m["negiota"] = -np.arange(128, dtype=np.float32).reshape(128, 1)
    m["halfpi"] = np.full((128, 1), math.pi / 2.0, np.float32)
    m["ident"] = np.eye(128, dtype=np.float32)
    m["ones_bf"] = np.ones((128, 1), np.float16)
    return {k: np.ascontiguousarray(np.asarray(v)) for k, v in m.items()}


def make_in_maps(inputs):
    inputs = {k: np.asarray(v) for k, v in inputs.items()}
    nwfc = [_neg_wywfc(np.asarray(inputs["W_fc"], np.float32), H)
            for H, W in LEVELS]
    in_maps = []
    for c in range(N_CORES):
        sl = slice(c * NB, (c + 1) * NB)
        inp_slice = {k: (v[sl] if k in ("feat0", "feat1", "feat2", "inputs", "t")
                         else v) for k, v in inputs.items()}
        in_maps.append(_host_inputs(inp_slice, nwfc))
    return in_maps


def kernel(**inputs):
    if "prog" not in _CACHE:
        _CACHE["prog"] = _build_program()
    nc = _CACHE["prog"]
    in_maps = make_in_maps(inputs)
    res = bass_utils.run_bass_kernel_spmd(nc, in_maps,
                                          core_ids=list(range(N_CORES)))
    out = np.concatenate([res.results[c]["out"] for c in range(N_CORES)], axis=0)
    return np.ascontiguousarray(out.astype(np.float32))



# revision 1
# speedup vs baseline: 2659.6890x; 2659.6890x over previous
"""Trainium2 Bass kernel for nn_CLRerHead (CLRNet-style lane-detection head).

Sharding: data-parallel over batch. 32 batch items -> 8 cores x 4 items each.
Each core runs the full 3-level refinement for its 4 items.

Gather strategy ("negative tent" matmul gather): per level,
  grid_sample + roi-flatten + FC fuse into two PE matmul stages:
    RCPROJ_s (W, 64) = sum_{corner} featRow_y(64, W).T @ (-wy*Wfc_s)(64, 64)
    f (64, 192)     += sum_s matmul(lhsT=RCPROJ_s(W, 64), rhs=v_s(W, 192))
  with v(W, 192*36) = min(|broadcast(xf) - iota| - 1, 0) = -(x-direction tent
  weight), built by gpsimd partition_broadcast + ACT Abs + DVE tensor_scalar.
  The two minus signs cancel.
"""

import math
import numpy as np
import ml_dtypes
from contextlib import ExitStack

import concourse.bass as bass
import concourse.bacc as bacc
import concourse.mybir as mybir
import concourse.tile as tile
from concourse import bass_utils

dt = mybir.dt
AF = mybir.ActivationFunctionType
ALU = mybir.AluOpType

# ---------------- static problem config ----------------
IMG_W, IMG_H = 800.0, 320.0
NR, NS, NP, FC = 72, 36, 192, 64
N_STRIPS = NR - 1
ALPHA = IMG_H / IMG_W
SAMPLE_IDX = (np.linspace(0.0, 1.0, NS) * N_STRIPS).astype(np.int64)
PRIOR_FEAT_YS = np.flip(SAMPLE_IDX.astype(np.float32) / N_STRIPS).copy()
PRIOR_YS = np.linspace(1.0, 0.0, NR, dtype=np.float32)

N_CORES = 8
NB = 4
LEVELS = [(10, 25), (20, 50), (40, 100)]   # processing order: feat2, feat1, feat0
PCH = [(0, 128), (128, 64)]
BF16 = dt.bfloat16
FP16 = dt.float16
F32 = dt.float32
NPTS = NP * NS
NHALF = NPTS // 2
NQ = NPTS // 4
SPQ = NS // 4
SPH = NS // 2
SG = 8

Q_S = (1.0 - PRIOR_YS[SAMPLE_IDX[::-1]]).astype(np.float32)
QF_R = (1.0 - PRIOR_YS).astype(np.float32)


def _level_ytab(H):
    ys = PRIOR_FEAT_YS * (H - 1)
    y0 = np.clip(np.floor(ys).astype(np.int64), 0, H - 1)
    y1 = np.minimum(y0 + 1, H - 1)
    wy1 = (ys - y0).astype(np.float32)
    wy1 = np.where(y1 == y0, 0.0, wy1).astype(np.float32)
    wy0 = (1.0 - wy1).astype(np.float32)
    return y0, y1, wy0, wy1


def _proj_descriptors(H):
    """[(corner, y, sa, sb, col, first, last)] with one start/stop per 8-s group."""
    y0, y1, _, _ = _level_ytab(H)
    raw = []
    for corner, yc in enumerate((y0, y1)):
        s = 0
        while s < NS:
            e = s
            while e + 1 < NS and yc[e + 1] == yc[s]:
                e += 1
            a = s
            while a <= e:
                b = min(e, (a // SG) * SG + SG - 1)
                raw.append((corner, int(yc[s]), a, b))
                a = b + 1
            s = e + 1
    raw.sort(key=lambda r: (r[2] // SG, r[0], r[2]))
    descs, col, seen, last_in_group = [], 0, set(), {}
    for i, (corner, y, sa, sb) in enumerate(raw):
        g = sa // SG
        descs.append([corner, y, sa, sb, col, g not in seen, False])
        seen.add(g)
        last_in_group[g] = i
        col += 64 * (sb - sa + 1)
    for i in last_in_group.values():
        descs[i][6] = True
    return descs, col


def _neg_wywfc(W_fc, H):
    _, _, wy0, wy1 = _level_ytab(H)
    wyc = [wy0, wy1]
    descs, ncols = _proj_descriptors(H)
    out = np.zeros((64, ncols), np.float32)
    for corner, y, sa, sb, col, _, _ in descs:
        for s in range(sa, sb + 1):
            out[:, col:col + 64] = -wyc[corner][s] * W_fc[s::NS, :]
            col += 64
    return out.astype(np.float16)


_CACHE = {}


def _build_program(num_devices=N_CORES):
    nc = bacc.Bacc("TRN2", target_bir_lowering=False, debug=False,
                   num_devices=num_devices)
    D = {}

    def din(name, shape, dtype=F32):
        D[name] = nc.dram_tensor(name, list(shape), dtype, kind="ExternalInput")

    for li, (H, W) in enumerate(LEVELS):
        din(f"featbf{li}", (64, NB * H * W), FP16)
        _, ncols = _proj_descriptors(H)
        din(f"nwfc{li}", (64, ncols), FP16)
    din("anch", (NB, NP, 3))
    din("sinargsT", (64, NB))
    din("W_t1", (64, 256)); din("b_t1", (128, 2))
    din("W_t2a", (128, 256)); din("W_t2b", (128, 256)); din("b_t2", (128, 2))
    din("W_sta", (128, 128)); din("W_stb", (128, 128))
    din("bstS1", (64, 1)); din("bstSh", (64, 1))
    din("W_tca", (128, 64)); din("W_tcb", (128, 64)); din("b_tc", (64, 1))
    for w in ["W_q", "W_k", "W_v", "W_c1", "W_c2", "W_r1", "W_r2"]:
        din(w, (64, 64))
    din("W_o_bf", (64, 64), FP16)
    din("W_cls", (64, 2)); din("W_reg", (64, 76))
    for bnm in ["b_fc", "b_c1", "b_c2", "b_r1", "b_r2"]:
        din(bnm, (64, 1))
    din("b_cls", (2, 1)); din("b_reg", (76, 1))
    din("qrep", (128, NS)); din("qfrep", (128, NR))
    din("negiota", (128, 1)); din("halfpi", (128, 1)); din("ident", (128, 128))
    din("ones_bf", (128, 1), FP16)

    out_t = nc.dram_tensor("out", [NB, NP, 78], F32, kind="ExternalOutput")

    with tile.TileContext(nc) as tc, ExitStack() as ex:
        cpool = ex.enter_context(tc.tile_pool(name="consts", bufs=1))
        state = ex.enter_context(tc.tile_pool(name="state", bufs=1))
        wk = ex.enter_context(tc.tile_pool(name="work", bufs=2))
        big = ex.enter_context(tc.tile_pool(name="big", bufs=2))
        ps = ex.enter_context(tc.tile_pool(name="ps", bufs=4, space="PSUM"))
        psf = ex.enter_context(tc.tile_pool(name="psf", bufs=2, space="PSUM"))
        psrc = ex.enter_context(tc.tile_pool(name="psrc", bufs=2, space="PSUM"))

        C = {}
        for name, t in D.items():
            if name == "anch":
                continue
            C[name] = cpool.tile(list(t.shape), t.dtype, tag=name, name=f"c_{name}")
            nc.sync.dma_start(C[name][:], t.ap())

        anch = {}
        for b in range(NB):
            for ci, (p0, pn) in enumerate(PCH):
                a = state.tile([pn, 3], F32, tag=f"anch{b}_{ci}", name=f"anch{b}_{ci}")
                nc.sync.dma_start(a[:], D["anch"].ap()[b, p0:p0 + pn, :])
                anch[(b, ci)] = a

        # ---------------- time MLP ----------------
        sinT = wk.tile([64, NB], F32, tag="tm_sin", name="sinT")
        nc.scalar.activation(sinT[:], C["sinargsT"][:], AF.Sin)
        emb = []
        for m in range(2):
            p = ps.tile([128, NB], F32, tag="mm", name=f"p_emb{m}")
            nc.tensor.matmul(p[:], C["W_t1"][:, m * 128:(m + 1) * 128], sinT[:])
            x = state.tile([128, NB], F32, tag=f"emb{m}", name=f"emb{m}")
            nc.scalar.activation(x[:], p[:], AF.Identity, bias=C["b_t1"][:, m:m + 1])
            sq = wk.tile([128, NB], F32, tag="tm_sq", name=f"sq{m}")
            nc.scalar.activation(sq[:], x[:], AF.Square)
            cu = wk.tile([128, NB], F32, tag="tm_cu", name=f"cu{m}")
            nc.vector.tensor_tensor(cu[:], sq[:], x[:], ALU.mult)
            nc.vector.tensor_scalar(cu[:], cu[:], 0.044715, None, ALU.mult)
            nc.vector.tensor_tensor(cu[:], cu[:], x[:], ALU.add)
            th = wk.tile([128, NB], F32, tag="tm_th", name=f"th{m}")
            nc.scalar.activation(th[:], cu[:], AF.Tanh,
                                 scale=float(np.sqrt(2.0 / np.pi)))
            nc.vector.tensor_scalar(th[:], th[:], 1.0, 0.5, ALU.add, ALU.mult)
            nc.vector.tensor_tensor(x[:], th[:], x[:], ALU.mult)
            emb.append(x)
        tmb = []
        for m in range(2):
            p = ps.tile([128, NB], F32, tag="mm", name=f"p_tmb{m}")
            for k in range(2):
                wt2 = C["W_t2a"] if k == 0 else C["W_t2b"]
                nc.tensor.matmul(p[:], wt2[:, m * 128:(m + 1) * 128], emb[k][:],
                                 start=(k == 0), stop=(k == 1))
            x = state.tile([128, NB], F32, tag=f"tmb{m}", name=f"tmb{m}")
            nc.scalar.activation(x[:], p[:], AF.Identity, bias=C["b_t2"][:, m:m + 1])
            tmb.append(x)
        sil = []
        for m in range(2):
            s = wk.tile([128, NB], F32, tag=f"tm_sil{m}", name=f"sil{m}")
            nc.scalar.activation(s[:], tmb[m][:], AF.Sigmoid)
            nc.vector.tensor_tensor(s[:], s[:], tmb[m][:], ALU.mult)
            sil.append(s)
        scale1T = state.tile([64, NB], F32, tag="scale1T", name="scale1T")
        shiftT = state.tile([64, NB], F32, tag="shiftT", name="shiftT")
        for j, (dst, bias) in enumerate([(scale1T, "bstS1"), (shiftT, "bstSh")]):
            p = ps.tile([64, NB], F32, tag="mm", name=f"p_ss{j}")
            for k in range(2):
                wst = C["W_sta"] if k == 0 else C["W_stb"]
                nc.tensor.matmul(p[:], wst[:, j * 64:(j + 1) * 64], sil[k][:],
                                 start=(k == 0), stop=(k == 1))
            nc.scalar.activation(dst[:], p[:], AF.Identity, bias=C[bias][:, 0:1])
        tokT = state.tile([64, NB], F32, tag="tokT", name="tokT")
        ptk = ps.tile([64, NB], F32, tag="mm", name="p_tok")
        for k in range(2):
            wtc = C["W_tca"] if k == 0 else C["W_tcb"]
            nc.tensor.matmul(ptk[:], wtc[:], tmb[k][:], start=(k == 0), stop=(k == 1))
        nc.scalar.activation(tokT[:], ptk[:], AF.Identity, bias=C["b_tc"][:, 0:1])

        # ---------------- helpers ----------------
        def gen_ab(b, scaleW, tagsfx):
            """Emit trig + affine ops; return [(aC, bC, base, g)] per chunk."""
            res = []
            for ci, (p0, pn) in enumerate(PCH):
                A = anch[(b, ci)]
                sn = wk.tile([pn, 1], F32, tag=f"sn{ci}{tagsfx}", name=f"sn{b}{ci}")
                cs = wk.tile([pn, 1], F32, tag=f"cs{ci}{tagsfx}", name=f"cs{b}{ci}")
                nc.scalar.activation(sn[:], A[:, 2:3], AF.Sin, scale=math.pi)
                nc.scalar.activation(cs[:], A[:, 2:3], AF.Sin, scale=-math.pi,
                                     bias=C["halfpi"][0:pn, 0:1])
                g = wk.tile([pn, 1], F32, tag=f"g{ci}{tagsfx}", name=f"g{b}{ci}")
                nc.vector.reciprocal(g[:], sn[:])
                nc.vector.tensor_tensor(g[:], cs[:], g[:], ALU.mult)
                nc.vector.tensor_scalar(g[:], g[:], 1000.0, -1000.0,
                                        ALU.min, ALU.max)
                nc.vector.tensor_scalar(g[:], g[:], ALPHA, None, ALU.mult)
                base = wk.tile([pn, 1], F32, tag=f"bs{ci}{tagsfx}", name=f"bs{b}{ci}")
                nc.vector.tensor_tensor(base[:], A[:, 0:1], g[:], ALU.mult)
                nc.vector.tensor_tensor(base[:], A[:, 1:2], base[:], ALU.subtract)
                aC = wk.tile([pn, 1], F32, tag=f"aC{ci}{tagsfx}", name=f"aC{b}{ci}")
                bC = wk.tile([pn, 1], F32, tag=f"bC{ci}{tagsfx}", name=f"bC{b}{ci}")
                nc.vector.tensor_scalar(aC[:], base[:], scaleW, None, ALU.mult)
                nc.vector.tensor_scalar(bC[:], g[:], scaleW, None, ALU.mult)
                res.append((aC, bC, base, g))
            return res

        # ---------------- main loop ----------------
        for li, (H, W) in enumerate(LEVELS):
            HW = H * W
            descs, _ = _proj_descriptors(H)
            is_last = li == len(LEVELS) - 1
            feat = C[f"featbf{li}"]
            nwfc = C[f"nwfc{li}"]

            # stage A (trig table) for all b
            xfs = {}
            for b in range(NB):
                ab = gen_ab(b, float(W - 1), "a")
                for ci, (p0, pn) in enumerate(PCH):
                    aC, bC, _, _ = ab[ci]
                    xf = state.tile([pn, NS], F32, tag=f"xf{b}_{ci}",
                                    name=f"xf{b}_{ci}_{li}")
                    nc.vector.tensor_scalar(xf[:], C["qrep"][0:pn, :],
                                            bC[:, 0:1], aC[:, 0:1],
                                            ALU.mult, ALU.add)
                    xfs[(b, ci)] = xf

            for b in range(NB):
                # xf -> row (1, 6912) via PE transpose + DMA collapse
                xfTs = wk.tile([NS, NP], F32, tag="xfT", bufs=3, name=f"xfT{b}")
                for ci, (p0, pn) in enumerate(PCH):
                    pt = ps.tile([NS, 128], F32, tag="mm", name=f"p_xfT{b}{ci}")
                    nc.tensor.transpose(pt[:, 0:pn], xfs[(b, ci)][:],
                                        C["ident"][0:pn, 0:pn])
                    nc.vector.tensor_copy(xfTs[:, p0:p0 + pn], pt[:, 0:pn])
                vq = []
                for h in range(4):
                    xfrow = wk.tile([1, NQ], F32, tag="xfrow", bufs=3,
                                    name=f"xfrow{b}{h}")
                    nc.sync.dma_start(xfrow[0:1, :], xfTs[h * SPQ:(h + 1) * SPQ, :])
                    xfB = big.tile([128, NQ], F32, tag="xfB", bufs=2,
                                   name=f"xfB{b}{h}")
                    nc.gpsimd.partition_broadcast(
                        xfB[0:W, :], xfrow[0:1, :],
                        channels=W)
                    d1 = big.tile([128, NQ], F32, tag="d1", bufs=2,
                                  name=f"d1{b}{h}")
                    nc.scalar.activation(d1[0:W, :], xfB[0:W, :], AF.Abs,
                                         bias=C["negiota"][0:W, 0:1])
                    v = big.tile([128, NQ], F32, tag="vq", bufs=3, name=f"v{b}{h}")
                    nc.vector.tensor_scalar(v[0:W, :], d1[0:W, :], 1.0, 0.0,
                                            ALU.subtract, ALU.min)
                    vq.append(v)

                # PROJ for all s-groups first (PE work independent of tents)
                fps = psf.tile([64, NP], F32, tag="f_ps", name=f"fps{b}")
                rcs = []
                for g0 in range(0, NS, SG):
                    rc = psrc.tile([128, SG * 64], F32, tag="rc_ps",
                                   name=f"rc{b}_{g0}")
                    for corner, y, sa, sb, col, first, last in descs:
                        if sa // SG != g0 // SG:
                            continue
                        n = (sb - sa + 1) * 64
                        nc.tensor.matmul(
                            rc[0:W, (sa - g0) * 64:(sa - g0) * 64 + n],
                            feat[:, b * HW + y * W: b * HW + (y + 1) * W],
                            nwfc[:, col:col + n],
                            start=first, stop=last)
                    ng = (min(g0 + SG, NS) - g0) * 64
                    rcsb = wk.tile([128, SG * 64], F32, tag="rcsb", bufs=6,
                                   name=f"rcsb{b}_{g0}")
                    nc.vector.tensor_copy(rcsb[0:W, 0:ng], rc[0:W, 0:ng])
                    rcs.append(rcsb)
                for s in range(NS):
                    g0 = (s // SG) * SG
                    h = s // SPQ
                    nc.tensor.matmul(
                        fps[:],
                        rcs[s // SG][0:W, (s - g0) * 64:(s - g0 + 1) * 64],
                        vq[h][0:W, (s - SPQ * h) * NP:(s - SPQ * h + 1) * NP],
                        start=(s == 0), stop=(s == NS - 1))

                fT = wk.tile([64, NP], F32, tag="fT", bufs=3, name=f"fT{b}")
                nc.scalar.activation(fT[:], fps[:], AF.Relu, bias=C["b_fc"][:, 0:1])
                nc.vector.tensor_scalar(fT[:], fT[:], tokT[:, b:b + 1], None,
                                        ALU.add)

                # attention
                qp = ps.tile([64, NP], F32, tag="mm", name=f"qp{b}")
                nc.tensor.matmul(qp[:], C["W_q"][:], fT[:])
                qT = wk.tile([64, NP], F32, tag="qT", name=f"qT{b}")
                nc.scalar.activation(qT[:], qp[:], AF.Copy, scale=0.125)
                kp = ps.tile([64, NP], F32, tag="mm", name=f"kp{b}")
                nc.tensor.matmul(kp[:], C["W_k"][:], fT[:])
                kT = wk.tile([64, NP], F32, tag="kT", name=f"kT{b}")
                nc.vector.tensor_copy(kT[:], kp[:])
                vn = []
                for ci, (p0, pn) in enumerate(PCH):
                    vp = ps.tile([128, 64], F32, tag="mm", name=f"vp{b}{ci}")
                    nc.tensor.matmul(vp[0:pn, :], fT[:, p0:p0 + pn], C["W_v"][:])
                    vt = wk.tile([pn, 64], FP16, tag=f"vn{ci}", name=f"vn{b}{ci}")
                    nc.vector.tensor_copy(vt[:], vp[0:pn, :])
                    vn.append(vt)
                est = []
                for ci, (p0, pn) in enumerate(PCH):
                    sp = ps.tile([128, NP], F32, tag="mm", name=f"sp{b}{ci}")
                    nc.tensor.matmul(sp[0:pn, :], kT[:, p0:p0 + pn], qT[:])
                    e = wk.tile([pn, NP], FP16, tag=f"est{ci}", name=f"est{b}{ci}")
                    nc.scalar.activation(e[:], sp[0:pn, :], AF.Exp)
                    est.append(e)
                zp = ps.tile([1, NP], F32, tag="mm", name=f"zp{b}")
                for ci, (p0, pn) in enumerate(PCH):
                    nc.tensor.matmul(zp[:], C["ones_bf"][0:pn, 0:1], est[ci][:],
                                     start=(ci == 0), stop=(ci == 1))
                rrow = wk.tile([1, NP], F32, tag="rrow", name=f"rrow{b}")
                nc.vector.reciprocal(rrow[:], zp[:])
                rbc = wk.tile([64, NP], F32, tag="rbc", name=f"rbc{b}")
                nc.gpsimd.partition_broadcast(rbc[:], rrow[0:1, :], channels=64)
                avp = ps.tile([64, NP], F32, tag="mm", name=f"avp{b}")
                for ci in range(2):
                    nc.tensor.matmul(avp[:], vn[ci][:], est[ci][:],
                                     start=(ci == 0), stop=(ci == 1))
                avsb = wk.tile([64, NP], FP16, tag="avsb", name=f"avsb{b}")
                nc.vector.tensor_copy(avsb[:], avp[:])
                opp = ps.tile([64, NP], F32, tag="mm", name=f"opp{b}")
                nc.tensor.matmul(opp[:], C["W_o_bf"][:], avsb[:])
                t1 = wk.tile([64, NP], F32, tag="attnt", name=f"t1{b}")
                nc.vector.tensor_tensor(t1[:], opp[:], rbc[:], ALU.mult)
                nc.vector.tensor_tensor(fT[:], fT[:], t1[:], ALU.add)

                # FiLM
                nc.vector.tensor_scalar(fT[:], fT[:], scale1T[:, b:b + 1],
                                        shiftT[:, b:b + 1], ALU.mult, ALU.add)

                def head_mm(wname, bias, src, relu=True, out_p=64):
                    p = ps.tile([128, NP], F32, tag="mm", name=f"p_{wname}{b}")
                    nc.tensor.matmul(p[0:out_p, :], C[wname][:], src[:])
                    o = wk.tile([out_p, NP], F32, tag=f"hd_{wname}",
                                name=f"{wname}o{b}")
                    nc.scalar.activation(o[:], p[0:out_p, :],
                                         AF.Relu if relu else AF.Identity,
                                         bias=C[bias][:, 0:1])
                    return o

                r1 = head_mm("W_r1", "b_r1", fT)
                r2 = head_mm("W_r2", "b_r2", r1)
                regT = head_mm("W_reg", "b_reg", r2, relu=False, out_p=76)

                for ci, (p0, pn) in enumerate(PCH):
                    pt = ps.tile([128, 76], F32, tag="mm", name=f"p_rt{b}{ci}")
                    nc.tensor.transpose(pt[0:pn, :], regT[:, p0:p0 + pn],
                                        C["ident"][0:76, 0:76])
                    rn = state.tile([pn, 76], F32, tag=f"regn{b}_{ci}",
                                    name=f"regn{b}_{ci}_{li}")
                    nc.vector.tensor_copy(rn[:], pt[0:pn, :])
                    A = anch[(b, ci)]
                    nc.vector.tensor_tensor(A[:, :], A[:, :], rn[:, 0:3], ALU.add)
                    if is_last:
                        o = out_t.ap()
                        nc.sync.dma_start(o[b, p0:p0 + pn, 2:5], A[:])
                        nc.sync.dma_start(o[b, p0:p0 + pn, 5:6], rn[:, 3:4])
                        _CACHE.setdefault("regn", {})[(b, ci)] = rn

                if is_last:
                    c1 = head_mm("W_c1", "b_c1", fT)
                    c2 = head_mm("W_c2", "b_c2", c1)
                    clsT = head_mm("W_cls", "b_cls", c2, relu=False, out_p=2)
                    for ci, (p0, pn) in enumerate(PCH):
                        pt = ps.tile([128, 2], F32, tag="mm", name=f"p_ct{b}{ci}")
                        nc.tensor.transpose(pt[0:pn, :], clsT[:, p0:p0 + pn],
                                            C["ident"][0:2, 0:2])
                        cn = wk.tile([pn, 2], F32, tag=f"clsn{ci}",
                                     name=f"clsn{b}{ci}")
                        nc.vector.tensor_copy(cn[:], pt[0:pn, :])
                        nc.sync.dma_start(out_t.ap()[b, p0:p0 + pn, 0:2], cn[:])

            if is_last:
                # pred_xs from updated anchors (batched so Sin acts group)
                for b in range(NB):
                    ab = gen_ab(b, 1.0, "o")
                    for ci, (p0, pn) in enumerate(PCH):
                        _, _, base, g = ab[ci]
                        rn = _CACHE["regn"][(b, ci)]
                        xsf = wk.tile([pn, NR], F32, tag=f"xsf{ci}",
                                      name=f"xsf{b}{ci}")
                        nc.vector.tensor_scalar(xsf[:], C["qfrep"][0:pn, :],
                                                g[:, 0:1], base[:, 0:1],
                                                ALU.mult, ALU.add)
                        nc.vector.tensor_tensor(xsf[:], xsf[:], rn[:, 4:76],
                                                ALU.add)
                        nc.sync.dma_start(out_t.ap()[b, p0:p0 + pn, 6:78], xsf[:])

    nc.compile()
    _CACHE.pop("regn", None)
    return nc


def _host_inputs(inp_slice, nwfc):
    m = {}
    for li, key in enumerate(["feat2", "feat1", "feat0"]):
        f = np.asarray(inp_slice[key], np.float32)
        H, W = LEVELS[li]
        fb = np.ascontiguousarray(f.transpose(1, 0, 2, 3).reshape(64, NB * H * W))
        m[f"featbf{li}"] = fb.astype(np.float16)
        m[f"nwfc{li}"] = nwfc[li]
    m["anch"] = np.asarray(inp_slice["inputs"], np.float32)
    half = FC // 2
    freqs = np.exp(np.arange(half, dtype=np.float32)
                   * (-math.log(10000.0) / (half - 1)))
    ang = np.asarray(inp_slice["t"]).astype(np.float32)[:, None] * freqs[None, :]
    full = np.concatenate([ang, ang + math.pi / 2.0], axis=1)
    full = np.mod(full + math.pi, 2.0 * math.pi) - math.pi
    m["sinargsT"] = np.ascontiguousarray(full.T).astype(np.float32)
    w = {k: np.asarray(v, np.float32) for k, v in inp_slice.items()
         if k.startswith(("W_", "b_"))}
    m["W_t1"] = w["W_t1"]
    m["b_t1"] = np.ascontiguousarray(w["b_t1"].reshape(2, 128).T)
    m["W_t2a"] = w["W_t2"][:128]; m["W_t2b"] = w["W_t2"][128:]
    m["b_t2"] = np.ascontiguousarray(w["b_t2"].reshape(2, 128).T)
    m["W_sta"] = w["W_st"][:128]; m["W_stb"] = w["W_st"][128:]
    m["bstS1"] = (w["b_st"][:64] + 1.0).reshape(-1, 1)
    m["bstSh"] = w["b_st"][64:].reshape(-1, 1)
    m["W_tca"] = w["W_tc"][:128]; m["W_tcb"] = w["W_tc"][128:]
    m["b_tc"] = w["b_tc"].reshape(-1, 1)
    for k in ["W_q", "W_k", "W_v", "W_c1", "W_c2", "W_r1", "W_r2",
              "W_cls", "W_reg"]:
        m[k] = w[k]
    m["W_o_bf"] = w["W_o"].astype(np.float16)
    for k in ["b_fc", "b_c1", "b_c2", "b_r1", "b_r2", "b_cls", "b_reg"]:
        m[k] = w[k].reshape(-1, 1)
    m["qrep"] = np.broadcast_to(Q_S[None, :], (128, NS)).copy()
    m["qfrep"] = np.broadcast_to(QF_R[None, :], (128, NR)).copy()
    m["negiota"] = -np.arange(128, dtype=np.float32).reshape(128, 1)
    m["halfpi"] = np.full((128, 1), math.pi / 2.0, np.float32)
    m["ident"] = np.eye(128, dtype=np.float32)
    m["ones_bf"] = np.ones((128, 1), np.float16)
    return {k: np.ascontiguousarray(np.asarray(v)) for k, v in m.items()}


def make_in_maps(inputs):
    inputs = {k: np.asarray(v) for k, v in inputs.items()}
    nwfc = [_neg_wywfc(np.asarray(inputs["W_fc"], np.float32), H)
            for H, W in LEVELS]
    in_maps = []
    for c in range(N_CORES):
        sl = slice(c * NB, (c + 1) * NB)
        inp_slice = {k: (v[sl] if k in ("feat0", "feat1", "feat2", "inputs", "t")
                         else v) for k, v in inputs.items()}
        in_maps.append(_host_inputs(inp_slice, nwfc))
    return in_maps


def kernel(**inputs):
    if "prog" not in _CACHE:
        _CACHE["prog"] = _build_program()
    nc = _CACHE["prog"]
    in_maps = make_in_maps(inputs)
    res = bass_utils.run_bass_kernel_spmd(nc, in_maps,
                                          core_ids=list(range(N_CORES)))
    out = np.concatenate([res.results[c]["out"] for c in range(N_CORES)], axis=0)
    return np.ascontiguousarray(out.astype(np.float32))

